# revision 4
# baseline (speedup 1.0000x reference)
"""Trainium2 Bass kernel for the spatial-attention layer.

Math (reference):
    fp = input_h @ f            [B, N, D]   N = 64*64 = 4096, D = 64
    gp = x @ g                  [B, N, D]
    s  = gp @ fp^T              [B, N, N]
    beta = softmax(s, -1)
    o  = beta @ input_h         [B, N, C2]
    out = gamma * o + x

Distribution: 8 cores, core c handles batch b = c // 2 and query rows
[half*2048, (half+1)*2048) with half = c % 2. Each core sees the full
4096 keys of its batch.

Per-core kernel strategy (all matmuls in float32r, ~1.5e-4 rel err):
  - Scores are computed TRANSPOSED: sT[m, n] = fp[m] . gp[n], tiled
    [128 keys x 512 queries]. p = exp(sT) then directly serves as the
    stationary operand (lhsT) of the output matmul against natural-layout
    h tiles -- no transpose of the attention matrix is ever needed.
  - Softmax denominators come from an all-ones [128,128] stationary
    matmul accumulated over key tiles (column sums of p); normalization
    (and gamma) is folded into p before the output matmul.
  - h / x need channel-on-partition copies only for the two input
    projections; those are built with PE transposes into small staging
    tiles and consumed immediately.
  - Flash-style tiling over the 2048 queries in 4 blocks of 512 keeps
    p at [4096, 512] fp32 (8.4 MB) in SBUF.
"""

import numpy as np

import concourse.bass as bass
import concourse.mybir as mybir
import concourse.tile as tile
from concourse import bacc
from concourse.bass_utils import run_bass_kernel_spmd
from concourse.masks import make_identity

F32 = mybir.dt.float32
F32R = mybir.dt.float32r

B, W, C, D = 4, 64, 512, 64
N = W * W                  # 4096 spatial positions (keys per batch)
NQ = N // 2                # 2048 queries per core
N_CORES = 8
MT = N // 128              # 32 key tiles
QB = 4                     # query blocks of 512
QT = NQ // 128             # 16 query tiles

EXP_FN = mybir.ActivationFunctionType.Exp


def build_nc():
    nc = bacc.Bacc(None)
    xh_d = nc.dram_tensor("xh", [NQ, C], F32, kind="ExternalInput")
    h_d = nc.dram_tensor("h", [N, C], F32, kind="ExternalInput")
    f_d = nc.dram_tensor("f", [C, D], F32, kind="ExternalInput")
    g_d = nc.dram_tensor("g", [C, D], F32, kind="ExternalInput")
    gamma_d = nc.dram_tensor("gamma", [1], F32, kind="ExternalInput")
    out_d = nc.dram_tensor("out", [NQ, C], F32, kind="ExternalOutput")

    with tile.TileContext(nc) as tc:
        with (
            tc.tile_pool(name="consts", bufs=1) as consts,
            tc.tile_pool(name="h_pool", bufs=MT) as h_pool,
            tc.tile_pool(name="p_pool", bufs=MT) as p_pool,
            tc.tile_pool(name="stage", bufs=8) as stage_pool,
            tc.tile_pool(name="xload", bufs=3) as xload,
            tc.tile_pool(name="recips", bufs=2) as recips,
            tc.tile_pool(name="outp", bufs=3) as outp,
            tc.tile_pool(name="psA", bufs=4, space="PSUM") as psA,
            tc.tile_pool(name="psBig", bufs=2, space="PSUM") as psBig,
            tc.tile_pool(name="psSum", bufs=2, space="PSUM") as psSum,
        ):
            # ---- constants -------------------------------------------------
            ident = consts.tile([128, 128], F32)
            make_identity(nc, ident)
            ident_r = consts.tile([128, 128], F32R)
            nc.vector.tensor_copy(ident_r, ident)

            ones_f = consts.tile([128, 128], F32)
            nc.vector.memset(ones_f, 1.0)
            ones_r = consts.tile([128, 128], F32R)
            nc.vector.tensor_copy(ones_r, ones_f)

            gamma_sb = consts.tile([128, 1], F32)
            nc.sync.dma_start(
                gamma_sb,
                bass.AP(tensor=gamma_d, offset=0, ap=[[0, 128], [1, 1]]),
            )

            # f, g: [512, 64] -> [128, 4k, 64] (channel k-tiles on partitions)
            f_sb = consts.tile([128, 4, D], F32R)
            g_sb = consts.tile([128, 4, D], F32R)
            nc.sync.dma_start(
                f_sb, f_d[:, :].rearrange("(k p) d -> p k d", p=128).bitcast(F32R)
            )
            nc.sync.dma_start(
                g_sb, g_d[:, :].rearrange("(k p) d -> p k d", p=128).bitcast(F32R)
            )

            # fpT [64, 4096] and gpT [64, 2048] packed into one tile:
            # proj[:, 0:4096] = fpT, proj[:, 4096:6144] = gpT
            proj = consts.tile([64, N + NQ], F32R)

            # ---- phase 1a: load h, build fpT = (h @ f)^T -------------------
            h_sb = []
            for mg in range(MT // 4):
                stg = [stage_pool.tile([128, 512], F32R, tag="stage",
                                       name=f"stg_h_{mg}_{k}")
                       for k in range(4)]
                for j in range(4):
                    m = mg * 4 + j
                    ht = h_pool.tile([128, C], F32R, tag="h")
                    h_sb.append(ht)
                    nc.sync.dma_start(
                        ht, h_d[m * 128:(m + 1) * 128, :].bitcast(F32R)
                    )
                    for k in range(4):
                        tp = psA.tile([128, 128], F32R, tag="psA")
                        nc.tensor.transpose(
                            tp, ht[:, k * 128:(k + 1) * 128], ident_r
                        )
                        nc.vector.tensor_copy(
                            stg[k][:, j * 128:(j + 1) * 128], tp
                        )
                fpp = psBig.tile([64, 512], F32, tag="psBig")
                for k in range(4):
                    nc.tensor.matmul(
                        fpp, f_sb[:, k, :], stg[k],
                        start=(k == 0), stop=(k == 3),
                    )
                nc.vector.tensor_copy(
                    proj[:, mg * 512:(mg + 1) * 512], fpp
                )

            # ---- phase 1b: stream x, build gpT = (x @ g)^T -----------------
            for ng in range(QT // 4):
                stg = [stage_pool.tile([128, 512], F32R, tag="stage",
                                       name=f"stg_x_{ng}_{k}")
                       for k in range(4)]
                for j in range(4):
                    n = ng * 4 + j
                    xt = xload.tile([128, C], F32, tag="xload")
                    nc.sync.dma_start(xt, xh_d[n * 128:(n + 1) * 128, :])
                    for k in range(4):
                        tp = psA.tile([128, 128], F32, tag="psA")
                        nc.tensor.transpose(
                            tp, xt[:, k * 128:(k + 1) * 128], ident
                        )
                        nc.vector.tensor_copy(
                            stg[k][:, j * 128:(j + 1) * 128], tp
                        )
                gpp = psBig.tile([64, 512], F32, tag="psBig")
                for k in range(4):
                    nc.tensor.matmul(
                        gpp, g_sb[:, k, :], stg[k],
                        start=(k == 0), stop=(k == 3),
                    )
                nc.vector.tensor_copy(
                    proj[:, N + ng * 512:N + (ng + 1) * 512], gpp
                )

            # ---- phase 2: attention over query blocks of 512 ---------------
            for nb in range(QB):
                q_cols = proj[:, N + nb * 512:N + (nb + 1) * 512]
                sums_ps = psSum.tile([128, 512], F32, tag="psSum")
                p_tiles = []
                for m in range(MT):
                    sps = psA.tile([128, 512], F32, tag="psA")
                    nc.tensor.matmul(
                        sps, proj[:, m * 128:(m + 1) * 128], q_cols,
                        start=True, stop=True,
                    )
                    pt = p_pool.tile([128, 512], F32R, tag="p")
                    p_tiles.append(pt)
                    nc.scalar.activation(pt, sps, EXP_FN)
                    nc.tensor.matmul(
                        sums_ps, ones_r, pt,
                        start=(m == 0), stop=(m == MT - 1),
                    )
                # recip = gamma / colsums, broadcast on all 128 partitions
                recip = recips.tile([128, 512], F32, tag="recip")
                nc.vector.reciprocal(recip, sums_ps)
                nc.vector.tensor_scalar_mul(recip, recip, gamma_sb)
                for m in range(MT):
                    nc.vector.tensor_mul(p_tiles[m], p_tiles[m], recip)
                # output matmul: o[nq, c] = sum_m p_norm[m, nq] * h[m, c]
                for nt in range(4):
                    ops = psBig.tile([128, C], F32, tag="psBig")
                    for m in range(MT):
                        nc.tensor.matmul(
                            ops, p_tiles[m][:, nt * 128:(nt + 1) * 128],
                            h_sb[m],
                            start=(m == 0), stop=(m == MT - 1),
                        )
                    n_idx = nb * 4 + nt
                    xres = xload.tile([128, C], F32, tag="xload")
                    nc.sync.dma_start(
                        xres, xh_d[n_idx * 128:(n_idx + 1) * 128, :]
                    )
                    out_sb = outp.tile([128, C], F32, tag="out")
                    nc.vector.tensor_add(out_sb, ops, xres)
                    nc.sync.dma_start(
                        out_d[n_idx * 128:(n_idx + 1) * 128, :], out_sb
                    )

    nc.finalize()
    return nc


_NC_CACHE = None


def kernel(x, input_h, f, g, gamma):
    global _NC_CACHE
    x = np.asarray(x, dtype=np.float32)
    input_h = np.asarray(input_h, dtype=np.float32)
    f2 = np.ascontiguousarray(np.asarray(f, dtype=np.float32).reshape(C, D))
    g2 = np.ascontiguousarray(np.asarray(g, dtype=np.float32).reshape(C, D))
    gam = np.ascontiguousarray(np.asarray(gamma, dtype=np.float32).reshape(1))

    x_flat = x.reshape(B, N, C)
    h_flat = input_h.reshape(B, N, C)

    in_maps = []
    for c in range(N_CORES):
        b, half = c // 2, c % 2
        in_maps.append({
            "xh": np.ascontiguousarray(x_flat[b, half * NQ:(half + 1) * NQ]),
            "h": np.ascontiguousarray(h_flat[b]),
            "f": f2,
            "g": g2,
            "gamma": gam,
        })

    if _NC_CACHE is None:
        _NC_CACHE = build_nc()
    res = run_bass_kernel_spmd(_NC_CACHE, in_maps, core_ids=list(range(N_CORES)))

    out = np.empty((B, N, C), dtype=np.float32)
    for c in range(N_CORES):
        b, half = c // 2, c % 2
        out[b, half * NQ:(half + 1) * NQ] = res.results[c]["out"]
    return out.reshape(B, W, W, C)


# revision 5
# speedup vs baseline: 1.1210x; 1.1210x over previous
"""Trainium2 Bass kernel for the spatial-attention layer.

Math (reference):
    fp = input_h @ f            [B, N, D]   N = 64*64 = 4096, D = 64
    gp = x @ g                  [B, N, D]
    s  = gp @ fp^T              [B, N, N]
    beta = softmax(s, -1)
    o  = beta @ input_h         [B, N, C2]
    out = gamma * o + x

Distribution: 8 cores, core c handles batch b = c // 2 and query rows
[half*2048, (half+1)*2048) with half = c % 2. Each core sees the full
4096 keys of its batch.

Per-core kernel strategy (all matmuls in float32r, ~1.5e-4 rel err):
  - Scores are computed TRANSPOSED: sT[m, n] = fp[m] . gp[n], tiled
    [128 keys x 512 queries]. p = exp(sT) then directly serves as the
    stationary operand (lhsT) of the output matmul against natural-layout
    h tiles -- no transpose of the attention matrix is ever needed.
  - Softmax denominators come from an all-ones [128,128] stationary
    matmul accumulated over key tiles (column sums of p); normalization
    (and gamma) is folded into p before the output matmul.
  - h / x need channel-on-partition copies only for the two input
    projections; those are built with PE transposes into small staging
    tiles and consumed immediately.
  - Flash-style tiling over the 2048 queries in 4 blocks of 512 keeps
    p at [4096, 512] fp32 (8.4 MB) in SBUF.
"""

import numpy as np

import concourse.bass as bass
import concourse.mybir as mybir
import concourse.tile as tile
from concourse import bacc
from concourse.bass_utils import run_bass_kernel_spmd
from concourse.masks import make_identity

F32 = mybir.dt.float32
F32R = mybir.dt.float32r
BF16 = mybir.dt.bfloat16

B, W, C, D = 4, 64, 512, 64
N = W * W                  # 4096 spatial positions (keys per batch)
NQ = N // 2                # 2048 queries per core
N_CORES = 8
MT = N // 128              # 32 key tiles
QB = 4                     # query blocks of 512
QT = NQ // 128             # 16 query tiles

EXP_FN = mybir.ActivationFunctionType.Exp


def build_nc():
    nc = bacc.Bacc(None)
    xh_d = nc.dram_tensor("xh", [NQ, C], F32, kind="ExternalInput")
    h_d = nc.dram_tensor("h", [N, C], F32, kind="ExternalInput")
    f_d = nc.dram_tensor("f", [C, D], F32, kind="ExternalInput")
    g_d = nc.dram_tensor("g", [C, D], F32, kind="ExternalInput")
    gamma_d = nc.dram_tensor("gamma", [1], F32, kind="ExternalInput")
    out_d = nc.dram_tensor("out", [NQ, C], F32, kind="ExternalOutput")

    with tile.TileContext(nc) as tc:
        with (
            tc.tile_pool(name="consts", bufs=1) as consts,
            tc.tile_pool(name="h_pool", bufs=MT) as h_pool,
            tc.tile_pool(name="hr_pool", bufs=8) as hr_pool,
            tc.tile_pool(name="p_pool", bufs=MT) as p_pool,
            tc.tile_pool(name="stage", bufs=8) as stage_pool,
            tc.tile_pool(name="xload", bufs=3) as xload,
            tc.tile_pool(name="recips", bufs=2) as recips,
            tc.tile_pool(name="outp", bufs=3) as outp,
            tc.tile_pool(name="psA", bufs=4, space="PSUM") as psA,
            tc.tile_pool(name="psBig", bufs=2, space="PSUM") as psBig,
            tc.tile_pool(name="psSum", bufs=2, space="PSUM") as psSum,
        ):
            # ---- constants -------------------------------------------------
            ident = consts.tile([128, 128], F32)
            make_identity(nc, ident)
            ident_r = consts.tile([128, 128], F32R)
            nc.vector.tensor_copy(ident_r, ident)

            ones_b = consts.tile([128, 128], BF16)
            nc.vector.memset(ones_b, 1.0)

            gamma_sb = consts.tile([128, 1], F32)
            nc.sync.dma_start(
                gamma_sb,
                bass.AP(tensor=gamma_d, offset=0, ap=[[0, 128], [1, 1]]),
            )

            # f, g: [512, 64] -> [128, 4k, 64] (channel k-tiles on partitions)
            f_sb = consts.tile([128, 4, D], F32R)
            g_sb = consts.tile([128, 4, D], F32R)
            nc.sync.dma_start(
                f_sb, f_d[:, :].rearrange("(k p) d -> p k d", p=128).bitcast(F32R)
            )
            nc.sync.dma_start(
                g_sb, g_d[:, :].rearrange("(k p) d -> p k d", p=128).bitcast(F32R)
            )

            # fpT [64, 4096] and gpT [64, 2048] packed into one tile:
            # proj[:, 0:4096] = fpT, proj[:, 4096:6144] = gpT
            proj = consts.tile([64, N + NQ], F32R)

            # ---- phase 1a: load h, build fpT = (h @ f)^T -------------------
            h_sb = []
            for mg in range(MT // 4):
                stg = [stage_pool.tile([128, 512], F32R, tag="stage",
                                       name=f"stg_h_{mg}_{k}")
                       for k in range(4)]
                for j in range(4):
                    m = mg * 4 + j
                    hb = h_pool.tile([128, C], BF16, tag="h")
                    h_sb.append(hb)
                    nc.gpsimd.dma_start(hb, h_d[m * 128:(m + 1) * 128, :])
                    ht = hr_pool.tile([128, C], F32R, tag="hr")
                    nc.sync.dma_start(
                        ht, h_d[m * 128:(m + 1) * 128, :].bitcast(F32R)
                    )
                    for k in range(4):
                        tp = psA.tile([128, 128], F32R, tag="psA")
                        nc.tensor.transpose(
                            tp, ht[:, k * 128:(k + 1) * 128], ident_r
                        )
                        nc.vector.tensor_copy(
                            stg[k][:, j * 128:(j + 1) * 128], tp
                        )
                fpp = psBig.tile([64, 512], F32, tag="psBig")
                for k in range(4):
                    nc.tensor.matmul(
                        fpp, f_sb[:, k, :], stg[k],
                        start=(k == 0), stop=(k == 3),
                    )
                nc.vector.tensor_copy(
                    proj[:, mg * 512:(mg + 1) * 512], fpp
                )

            # ---- phase 1b: stream x, build gpT = (x @ g)^T -----------------
            for ng in range(QT // 4):
                stg = [stage_pool.tile([128, 512], F32R, tag="stage",
                                       name=f"stg_x_{ng}_{k}")
                       for k in range(4)]
                for j in range(4):
                    n = ng * 4 + j
                    xt = xload.tile([128, C], F32, tag="xload")
                    nc.sync.dma_start(xt, xh_d[n * 128:(n + 1) * 128, :])
                    for k in range(4):
                        tp = psA.tile([128, 128], F32, tag="psA")
                        nc.tensor.transpose(
                            tp, xt[:, k * 128:(k + 1) * 128], ident
                        )
                        nc.vector.tensor_copy(
                            stg[k][:, j * 128:(j + 1) * 128], tp
                        )
                gpp = psBig.tile([64, 512], F32, tag="psBig")
                for k in range(4):
                    nc.tensor.matmul(
                        gpp, g_sb[:, k, :], stg[k],
                        start=(k == 0), stop=(k == 3),
                    )
                nc.vector.tensor_copy(
                    proj[:, N + ng * 512:N + (ng + 1) * 512], gpp
                )

            # ---- phase 2: attention over query blocks of 512 ---------------
            for nb in range(QB):
                q_cols = proj[:, N + nb * 512:N + (nb + 1) * 512]
                sums_ps = psSum.tile([128, 512], F32, tag="psSum")
                p_tiles = []
                for m in range(MT):
                    sps = psA.tile([128, 512], F32, tag="psA")
                    nc.tensor.matmul(
                        sps, proj[:, m * 128:(m + 1) * 128], q_cols,
                        start=True, stop=True,
                    )
                    pt = p_pool.tile([128, 512], BF16, tag="p")
                    p_tiles.append(pt)
                    nc.scalar.activation(pt, sps, EXP_FN)
                    nc.tensor.matmul(
                        sums_ps, ones_b, pt,
                        start=(m == 0), stop=(m == MT - 1),
                    )
                # recip = gamma / colsums, broadcast on all 128 partitions
                recip = recips.tile([128, 512], F32, tag="recip")
                nc.vector.reciprocal(recip, sums_ps)
                nc.vector.tensor_scalar_mul(recip, recip, gamma_sb)
                for m in range(MT):
                    nc.vector.tensor_mul(p_tiles[m], p_tiles[m], recip)
                # output matmul: o[nq, c] = sum_m p_norm[m, nq] * h[m, c]
                for nt in range(4):
                    ops = psBig.tile([128, C], F32, tag="psBig")
                    for m in range(MT):
                        nc.tensor.matmul(
                            ops, p_tiles[m][:, nt * 128:(nt + 1) * 128],
                            h_sb[m],
                            start=(m == 0), stop=(m == MT - 1),
                        )
                    n_idx = nb * 4 + nt
                    xres = xload.tile([128, C], F32, tag="xload")
                    nc.sync.dma_start(
                        xres, xh_d[n_idx * 128:(n_idx + 1) * 128, :]
                    )
                    out_sb = outp.tile([128, C], F32, tag="out")
                    nc.vector.tensor_add(out_sb, ops, xres)
                    nc.sync.dma_start(
                        out_d[n_idx * 128:(n_idx + 1) * 128, :], out_sb
                    )

    nc.finalize()
    return nc


_NC_CACHE = None


def kernel(x, input_h, f, g, gamma):
    global _NC_CACHE
    x = np.asarray(x, dtype=np.float32)
    input_h = np.asarray(input_h, dtype=np.float32)
    f2 = np.ascontiguousarray(np.asarray(f, dtype=np.float32).reshape(C, D))
    g2 = np.ascontiguousarray(np.asarray(g, dtype=np.float32).reshape(C, D))
    gam = np.ascontiguousarray(np.asarray(gamma, dtype=np.float32).reshape(1))

    x_flat = x.reshape(B, N, C)
    h_flat = input_h.reshape(B, N, C)

    in_maps = []
    for c in range(N_CORES):
        b, half = c // 2, c % 2
        in_maps.append({
            "xh": np.ascontiguousarray(x_flat[b, half * NQ:(half + 1) * NQ]),
            "h": np.ascontiguousarray(h_flat[b]),
            "f": f2,
            "g": g2,
            "gamma": gam,
        })

    if _NC_CACHE is None:
        _NC_CACHE = build_nc()
    res = run_bass_kernel_spmd(_NC_CACHE, in_maps, core_ids=list(range(N_CORES)))

    out = np.empty((B, N, C), dtype=np.float32)
    for c in range(N_CORES):
        b, half = c // 2, c % 2
        out[b, half * NQ:(half + 1) * NQ] = res.results[c]["out"]
    return out.reshape(B, W, W, C)


# revision 7
# speedup vs baseline: 1.3338x; 1.1899x over previous
"""Trainium2 Bass kernel for the spatial-attention layer.

Math (reference):
    fp = input_h @ f            [B, N, D]   N = 64*64 = 4096, D = 64
    gp = x @ g                  [B, N, D]
    s  = gp @ fp^T              [B, N, N]
    beta = softmax(s, -1)
    o  = beta @ input_h         [B, N, C2]
    out = gamma * o + x

Distribution: 8 cores, core c handles batch b = c // 2 and query rows
[half*2048, (half+1)*2048) with half = c % 2. Each core sees the full
4096 keys of its batch.

Per-core strategy:
  - Scores are computed TRANSPOSED, sT[m, n] = fp[m] . gp[n], in
    [128 keys x 512 queries] tiles, float32r (~1.5e-4), two K=64
    matmuls packed into PE row-groups (0,0)/(64,0) concurrently.
    The projections fpT/gpT are replicated on partitions 0-63 and
    64-127 to feed both row groups.
  - p = exp(sT) (bf16) then directly serves as the stationary operand
    of the output matmul against natural-layout bf16 h tiles; softmax
    denominators come from an all-ones stationary matmul, and the
    1/denominator (with gamma folded in) is applied to the *output*
    tiles in natural orientation, transposing the denominator row with
    a PE transpose first.
  - h / x channel-on-partition copies (for the projections) are built
    with PE transposes batched 4-per-PSUM-bank.
  - Flash tiling over queries in 4 blocks of 512; p is [4096, 512]
    bf16 (4.2 MB) per block.
"""

import numpy as np

import concourse.bass as bass
import concourse.mybir as mybir
import concourse.tile as tile
from concourse import bacc
from concourse.bass_utils import run_bass_kernel_spmd
from concourse.masks import make_identity

F32 = mybir.dt.float32
F32R = mybir.dt.float32r
BF16 = mybir.dt.bfloat16
MULT = mybir.AluOpType.mult
ADD = mybir.AluOpType.add

B, W, C, D = 4, 64, 512, 64
N = W * W                  # 4096 spatial positions (keys per batch)
NQ = N // 2                # 2048 queries per core
N_CORES = 8
MT = N // 128              # 32 key tiles
QB = 4                     # query blocks of 512
QT = NQ // 128             # 16 query tiles

EXP_FN = mybir.ActivationFunctionType.Exp


def build_nc():
    nc = bacc.Bacc(None)
    xh_d = nc.dram_tensor("xh", [NQ, C], F32, kind="ExternalInput")
    h_d = nc.dram_tensor("h", [N, C], F32, kind="ExternalInput")
    f_d = nc.dram_tensor("f", [C, D], F32, kind="ExternalInput")
    g_d = nc.dram_tensor("g", [C, D], F32, kind="ExternalInput")
    gamma_d = nc.dram_tensor("gamma", [1], F32, kind="ExternalInput")
    out_d = nc.dram_tensor("out", [NQ, C], F32, kind="ExternalOutput")

    with tile.TileContext(nc) as tc:
        with (
            tc.tile_pool(name="consts", bufs=1) as consts,
            tc.tile_pool(name="h_pool", bufs=MT) as h_pool,
            tc.tile_pool(name="hr_pool", bufs=8) as hr_pool,
            tc.tile_pool(name="p_pool", bufs=MT // 2) as p_pool,
            tc.tile_pool(name="stage", bufs=8) as stage_pool,
            tc.tile_pool(name="xload", bufs=6) as xload,
            tc.tile_pool(name="sums", bufs=2) as sums_pool,
            tc.tile_pool(name="scales", bufs=8) as scales,
            tc.tile_pool(name="outp", bufs=3) as outp,
            tc.tile_pool(name="psA", bufs=2, space="PSUM") as psA,
            tc.tile_pool(name="psB", bufs=2, space="PSUM") as psB,
            tc.tile_pool(name="psS", bufs=2, space="PSUM") as psS,
        ):
            # ---- constants -------------------------------------------------
            ident = consts.tile([128, 128], F32)
            make_identity(nc, ident)
            ident_r = consts.tile([128, 128], F32R)
            nc.vector.tensor_copy(ident_r, ident)

            ones_b = consts.tile([128, 128], BF16)
            nc.vector.memset(ones_b, 1.0)

            gamma_sb = consts.tile([128, 1], F32)
            nc.sync.dma_start(
                gamma_sb,
                bass.AP(tensor=gamma_d, offset=0, ap=[[0, 128], [1, 1]]),
            )

            # f, g: [512, 64] -> [128, 4k, 64] (channel k-tiles on partitions)
            f_sb = consts.tile([128, 4, D], F32R)
            g_sb = consts.tile([128, 4, D], F32R)
            nc.sync.dma_start(
                f_sb, f_d[:, :].rearrange("(k p) d -> p k d", p=128).bitcast(F32R)
            )
            nc.sync.dma_start(
                g_sb, g_d[:, :].rearrange("(k p) d -> p k d", p=128).bitcast(F32R)
            )

            # fpT [64, 4096] + gpT [64, 2048] packed side by side, and
            # replicated on partitions 0-63 / 64-127 for PE row-group packing.
            proj = consts.tile([128, N + NQ], F32R)

            # ---- phase 1a: load h, build fpT = (h @ f)^T -------------------
            h_sb = []
            for mg in range(MT // 4):
                stg = [stage_pool.tile([128, 512], F32R, tag="stage",
                                       name=f"stg_h_{mg}_{k}")
                       for k in range(4)]
                hts = []
                for j in range(4):
                    m = mg * 4 + j
                    hb = h_pool.tile([128, C], BF16, tag="h")
                    h_sb.append(hb)
                    nc.gpsimd.dma_start(hb, h_d[m * 128:(m + 1) * 128, :])
                    ht = hr_pool.tile([128, C], F32R, tag="hr")
                    hts.append(ht)
                    nc.sync.dma_start(
                        ht, h_d[m * 128:(m + 1) * 128, :].bitcast(F32R)
                    )
                for k in range(4):
                    tpm = psA.tile([128, 1024], F32R, tag="psA")
                    for j in range(4):
                        nc.tensor.transpose(
                            tpm[:, j * 128:(j + 1) * 128],
                            hts[j][:, k * 128:(k + 1) * 128], ident_r,
                        )
                    nc.vector.tensor_copy(stg[k], tpm[:, 0:512])
                fpp = psB.tile([64, 512], F32, tag="psB")
                for k in range(4):
                    nc.tensor.matmul(
                        fpp, f_sb[:, k, :], stg[k],
                        start=(k == 0), stop=(k == 3),
                    )
                nc.vector.tensor_copy(proj[0:64, mg * 512:(mg + 1) * 512], fpp)
                nc.vector.tensor_copy(proj[64:128, mg * 512:(mg + 1) * 512], fpp)

            # ---- phase 1b: stream x, build gpT = (x @ g)^T -----------------
            for ng in range(QT // 4):
                stg = [stage_pool.tile([128, 512], F32R, tag="stage",
                                       name=f"stg_x_{ng}_{k}")
                       for k in range(4)]
                xts = []
                for j in range(4):
                    n = ng * 4 + j
                    xt = xload.tile([128, C], F32, tag="xload")
                    xts.append(xt)
                    nc.sync.dma_start(xt, xh_d[n * 128:(n + 1) * 128, :])
                for k in range(4):
                    tpm = psA.tile([128, 1024], F32, tag="psA")
                    for j in range(4):
                        nc.tensor.transpose(
                            tpm[:, j * 128:(j + 1) * 128],
                            xts[j][:, k * 128:(k + 1) * 128], ident,
                        )
                    nc.vector.tensor_copy(stg[k], tpm[:, 0:512])
                gpp = psB.tile([64, 512], F32, tag="psB")
                for k in range(4):
                    nc.tensor.matmul(
                        gpp, g_sb[:, k, :], stg[k],
                        start=(k == 0), stop=(k == 3),
                    )
                nc.vector.tensor_copy(
                    proj[0:64, N + ng * 512:N + (ng + 1) * 512], gpp)
                nc.vector.tensor_copy(
                    proj[64:128, N + ng * 512:N + (ng + 1) * 512], gpp)

            # ---- phase 2: attention over query blocks of 512 ---------------
            for nb in range(QB):
                q_lo = proj[0:64, N + nb * 512:N + (nb + 1) * 512]
                q_hi = proj[64:128, N + nb * 512:N + (nb + 1) * 512]
                sums_ps = psS.tile([128, 512], F32, tag="psS")
                p_tiles = []
                for mi in range(MT // 2):
                    m0, m1 = 2 * mi, 2 * mi + 1
                    sps = psA.tile([128, 1024], F32, tag="psA")
                    nc.tensor.matmul(
                        sps[:, 0:512],
                        proj[0:64, m0 * 128:(m0 + 1) * 128], q_lo,
                        start=True, stop=True, tile_position=(0, 0),
                    )
                    nc.tensor.matmul(
                        sps[:, 512:1024],
                        proj[64:128, m1 * 128:(m1 + 1) * 128], q_hi,
                        start=True, stop=True, tile_position=(64, 0),
                    )
                    pt = p_pool.tile([128, 1024], BF16, tag="p")
                    p_tiles.append(pt)
                    nc.scalar.activation(pt, sps, EXP_FN)
                    nc.tensor.matmul(
                        sums_ps, ones_b, pt[:, 0:512],
                        start=(mi == 0), stop=False,
                    )
                    nc.tensor.matmul(
                        sums_ps, ones_b, pt[:, 512:1024],
                        start=False, stop=(mi == MT // 2 - 1),
                    )
                # per-query scale = gamma / colsum, in [128, 1] natural layout
                sums_sb = sums_pool.tile([128, 512], F32, tag="sums")
                nc.vector.tensor_copy(sums_sb, sums_ps)
                nt_scales = []
                for nt in range(4):
                    stp = psB.tile([128, 128], F32, tag="psB")
                    nc.tensor.transpose(
                        stp, sums_sb[:, nt * 128:(nt + 1) * 128], ident)
                    sc = scales.tile([128, 1], F32, tag="scale")
                    nt_scales.append(sc)
                    nc.vector.reciprocal(sc, stp[:, 0:1])
                    nc.vector.tensor_mul(sc, sc, gamma_sb)
                # output matmul: o[nq, c] = sum_m p[m, nq] * h[m, c]
                for nt in range(4):
                    ops = psB.tile([128, C], F32, tag="psB")
                    for mi in range(MT // 2):
                        for half in range(2):
                            m = 2 * mi + half
                            nc.tensor.matmul(
                                ops,
                                p_tiles[mi][:, half * 512 + nt * 128:
                                            half * 512 + (nt + 1) * 128],
                                h_sb[m],
                                start=(m == 0), stop=(m == MT - 1),
                            )
                    n_idx = nb * 4 + nt
                    xres = xload.tile([128, C], F32, tag="xload")
                    nc.sync.dma_start(
                        xres, xh_d[n_idx * 128:(n_idx + 1) * 128, :])
                    out_sb = outp.tile([128, C], F32, tag="out")
                    nc.vector.scalar_tensor_tensor(
                        out_sb, ops, nt_scales[nt], xres, op0=MULT, op1=ADD)
                    nc.sync.dma_start(
                        out_d[n_idx * 128:(n_idx + 1) * 128, :], out_sb)

    nc.finalize()
    return nc


_NC_CACHE = None


def kernel(x, input_h, f, g, gamma):
    global _NC_CACHE
    x = np.asarray(x, dtype=np.float32)
    input_h = np.asarray(input_h, dtype=np.float32)
    f2 = np.ascontiguousarray(np.asarray(f, dtype=np.float32).reshape(C, D))
    g2 = np.ascontiguousarray(np.asarray(g, dtype=np.float32).reshape(C, D))
    gam = np.ascontiguousarray(np.asarray(gamma, dtype=np.float32).reshape(1))

    x_flat = x.reshape(B, N, C)
    h_flat = input_h.reshape(B, N, C)

    in_maps = []
    for c in range(N_CORES):
        b, half = c // 2, c % 2
        in_maps.append({
            "xh": np.ascontiguousarray(x_flat[b, half * NQ:(half + 1) * NQ]),
            "h": np.ascontiguousarray(h_flat[b]),
            "f": f2,
            "g": g2,
            "gamma": gam,
        })

    if _NC_CACHE is None:
        _NC_CACHE = build_nc()
    res = run_bass_kernel_spmd(_NC_CACHE, in_maps, core_ids=list(range(N_CORES)))

    out = np.empty((B, N, C), dtype=np.float32)
    for c in range(N_CORES):
        b, half = c // 2, c % 2
        out[b, half * NQ:(half + 1) * NQ] = res.results[c]["out"]
    return out.reshape(B, W, W, C)
